# revision 41
# baseline (speedup 1.0000x reference)
"""AMEncoder (6-layer linear-attention transformer) on 8 TRN2 NeuronCores.

Sharding: sequence-parallel. Each core handles 512 of the 4096 sequence
positions (x both batch elements = 1024 token rows). Parameters are
replicated. The only cross-core communication is one AllReduce per layer
per batch element of the per-head-pair linear-attention state
M = K^T V (128x512 f32 = 256KB), overlapped with the Q projection.

Layout: the residual stream lives feature-major ht[D partitions, tokens]
so every linear is a weight-stationary matmul with no activation
transposes. K/V are produced token-major (rows on partitions) because
M = K^T V contracts over tokens. Weights are pre-transposed on the host
so all DMA loads are natural row loads. Big matmuls run in float16
(full PE rate + fast weight load); norm/stat matmuls run in float32r.
Epilogues run on wide [128,1024] two-bank PSUM tiles to halve the
elementwise op count.
"""

import os
from contextlib import ExitStack

import numpy as np

import concourse.bass as bass
import concourse.bacc as bacc
import concourse.tile as tile
import concourse.mybir as mybir
from concourse.bass_utils import run_bass_kernel_spmd

FP = mybir.dt.float32
FR = mybir.dt.float32r
FH = mybir.dt.float16
AF = mybir.ActivationFunctionType
ALU = mybir.AluOpType

D, H, FF, V, B, S = 512, 8, 2048, 4096, 2, 4096
NCORES = 8
SC = S // NCORES          # sequence positions per core
T = B * SC                # token rows per core
DC = D // 128             # feature chunks
FFC = FF // 128
VC = V // 128
DH = D // H               # head dim = 64
EPS = 1e-5

# bias_cols column layout: bq(0:4) bo(4:8) b2(8:12) g2(12:16) be2(16:20) b1(20:36)
COL_BQ, COL_BO, COL_B2, COL_G2, COL_BE2, COL_B1 = 0, 4, 8, 12, 16, 20
N_BCOLS = 36


def build(n_layers):
    nc = bacc.Bacc("TRN2", target_bir_lowering=False, debug=False,
                   num_devices=NCORES)
    L = n_layers

    h0 = nc.dram_tensor("h0", [DC, 128, T], FH, kind="ExternalInput").ap()
    wqkvo = nc.dram_tensor("wqkvo", [L, DC, 128, 4 * D], FH, kind="ExternalInput").ap()
    w1 = nc.dram_tensor("w1", [L, DC, 128, FF], FH, kind="ExternalInput").ap()
    w2 = nc.dram_tensor("w2", [L, FFC, 128, D], FH, kind="ExternalInput").ap()
    wfc = nc.dram_tensor("wfc", [V // 512, DC, 128, 512], FH, kind="ExternalInput").ap()
    bias_cols = nc.dram_tensor("bias_cols", [L, 128, N_BCOLS], FP, kind="ExternalInput").ap()
    bias_rows = nc.dram_tensor("bias_rows", [L, 1, 2 * D], FH, kind="ExternalInput").ap()
    bfc_cols = nc.dram_tensor("bfc_cols", [128, VC], FP, kind="ExternalInput").ap()
    cblk2 = nc.dram_tensor("cblk2", [128, 2], FR, kind="ExternalInput").ap()
    cblk2t = nc.dram_tensor("cblk2t", [2, 128], FR, kind="ExternalInput").ap()
    cones1 = nc.dram_tensor("cones1", [1, 128], FH, kind="ExternalInput").ap()
    cones1r = nc.dram_tensor("cones1r", [1, 128], FR, kind="ExternalInput").ap()
    cinvd = nc.dram_tensor("cinvd", [128, 1], FH, kind="ExternalInput").ap()
    ceps = nc.dram_tensor("ceps", [1, 128], FP, kind="ExternalInput").ap()
    cmask = nc.dram_tensor("cmask", [128, 128], FP, kind="ExternalInput").ap()
    out = nc.dram_tensor("out", [B, V, SC], FP, kind="ExternalOutput").ap()

    with tile.TileContext(nc) as tc, ExitStack() as ctx:
        constp = ctx.enter_context(tc.tile_pool(name="const", bufs=1))
        pwqkvo = ctx.enter_context(tc.tile_pool(name="wqkvo", bufs=2))
        pw1 = ctx.enter_context(tc.tile_pool(name="w1", bufs=1))
        pw2 = ctx.enter_context(tc.tile_pool(name="w2", bufs=1))
        pwfc = ctx.enter_context(tc.tile_pool(name="wfc", bufs=2))
        pbias = ctx.enter_context(tc.tile_pool(name="bias", bufs=2))
        pacts = ctx.enter_context(tc.tile_pool(name="acts", bufs=5))
        pscr = ctx.enter_context(tc.tile_pool(name="scr", bufs=4))
        pfsb = ctx.enter_context(tc.tile_pool(name="fsb", bufs=2))
        pfsall = ctx.enter_context(tc.tile_pool(name="fsall", bufs=1))
        pkv = ctx.enter_context(tc.tile_pool(name="kv", bufs=4))
        psmall = ctx.enter_context(tc.tile_pool(name="small", bufs=4))
        pstat = ctx.enter_context(tc.tile_pool(name="stat", bufs=2))
        pmst = ctx.enter_context(tc.tile_pool(name="mst", bufs=2))
        psW = ctx.enter_context(tc.tile_pool(name="psW", bufs=3, space="PSUM"))
        psN = ctx.enter_context(tc.tile_pool(name="psN", bufs=2, space="PSUM"))
        pdram = ctx.enter_context(tc.tile_pool(name="dram", bufs=4, space="DRAM"))

        # --- constants (DMA'd from host; tiny DMAs must be >= 512B) ---
        ones1 = constp.tile([1, 128], FH, tag="c_ones1")
        nc.sync.dma_start(ones1[:], cones1[:])
        ones1r = constp.tile([1, 128], FR, tag="c_ones1r")
        nc.sync.dma_start(ones1r[:], cones1r[:])
        invD = constp.tile([128, 1], FH, tag="c_invD")
        nc.sync.dma_start(invD[:], cinvd[:])
        blk2 = constp.tile([128, 2], FR, tag="c_blk2")
        nc.sync.dma_start(blk2[:], cblk2[:])
        blk2t = constp.tile([2, 128], FR, tag="c_blk2t")
        nc.sync.dma_start(blk2t[:], cblk2t[:])
        epsc = constp.tile([1, 128], FP, tag="c_eps")
        nc.sync.dma_start(epsc[:], ceps[:])
        maskc = constp.tile([128, 128], FP, tag="c_mask")
        nc.sync.dma_start(maskc[:], cmask[:])

        # --- initial activations ---
        ht = pacts.tile([128, DC, T], FH, tag="act")
        for dc in range(DC):
            nc.sync.dma_start(ht[:, dc, :], h0[dc])

        for l in range(L):
            # ---- weight / bias loads for this layer ----
            wt = pwqkvo.tile([128, DC, 4 * D], FH, tag="wqkvo")
            for dc in range(DC):
                nc.sync.dma_start(wt[:, dc, :], wqkvo[l, dc])
            bcol = pbias.tile([128, N_BCOLS], FP, tag="bcol")
            nc.sync.dma_start(bcol[:], bias_cols[l])
            brow = pbias.tile([1, 2 * D], FH, tag="brow")
            nc.sync.dma_start(brow[:], bias_rows[l])

            # ---- K/V (token-major, rc-pairs) + unitelu(K) + M = K^T V ----
            msb = [pmst.tile([128, D], FP, tag="msb", name=f"msb{l}_{b}")
                   for b in range(B)]
            mar = [None, None]
            for pr in range(4):                     # rc pair (2pr, 2pr+1)
                b = pr // 2
                psK = psW.tile([128, 2, D], FP, tag="w", name="psK")
                psV = psW.tile([128, 2, D], FP, tag="w", name="psV")
                for hf in range(2):
                    rc = 2 * pr + hf
                    for which, ps in ((1, psK), (2, psV)):
                        for dc in range(DC):
                            nc.tensor.matmul(
                                ps[:, hf, :],
                                ht[:, dc, rc * 128:(rc + 1) * 128],
                                wt[:, dc, which * D:(which + 1) * D],
                                start=(dc == 0), stop=False,
                            )
                        nc.tensor.matmul(             # +bias (rank-1)
                            ps[:, hf, :], ones1[:],
                            brow[:, (which - 1) * D:which * D],
                            start=False, stop=True,
                        )
                vn = pkv.tile([128, 2, D], FH, tag="kv")
                nc.scalar.activation(vn[:], psV[:], AF.Copy)
                # unitelu over each head's 64 columns
                sq = pscr.tile([128, 2, D], FP, tag="scr")
                nc.scalar.activation(sq[:], psK[:], AF.Square)
                ss = psmall.tile([128, 2 * H], FP, tag="ss")
                nc.vector.tensor_reduce(
                    ss[:], sq[:].rearrange("p t (h d) -> p (t h) d", h=H),
                    axis=mybir.AxisListType.X, op=ALU.add)
                nrm = psmall.tile([128, 2 * H], FP, tag="nrm")
                nc.scalar.activation(nrm[:], ss[:], AF.Sqrt)
                ninv = psmall.tile([128, 2 * H], FP, tag="ninv")
                nc.vector.reciprocal(ninv[:], nrm[:])
                u = pkv.tile([128, 2, D], FH, tag="kv")
                nc.vector.tensor_tensor(
                    u[:].rearrange("p t (h d) -> p (t h) d", h=H),
                    psK[:].rearrange("p t (h d) -> p (t h) d", h=H),
                    ninv[:].broadcast_to([128, 2 * H, DH]),
                    op=ALU.mult)
                # elu(u) = (max(u,0) - 1) + min(exp(u), 1)
                a = pkv.tile([128, 2, D], FH, tag="kv")
                nc.vector.tensor_scalar(a[:], u[:], 0.0, -1.0, ALU.max, ALU.add)
                e = pkv.tile([128, 2, D], FH, tag="kv")
                nc.scalar.activation(e[:], u[:], AF.Exp)
                e1 = pkv.tile([128, 2, D], FH, tag="kv")
                nc.vector.tensor_scalar_min(e1[:], e[:], 1.0)
                kn = pkv.tile([128, 2, D], FH, tag="kv")
                nc.vector.tensor_tensor(kn[:], a[:], e1[:], op=ALU.add)
                for hf in range(2):
                    mrc = psN.tile([128, D], FP, tag="n", name="mrc")
                    for pair in range(4):
                        nc.tensor.matmul(
                            mrc[:, pair * 128:(pair + 1) * 128],
                            kn[:, hf, pair * 128:(pair + 1) * 128],
                            vn[:, hf, pair * 128:(pair + 1) * 128],
                            start=True, stop=True,
                        )
                    if pr % 2 == 0 and hf == 0:
                        nc.vector.tensor_copy(msb[b][:], mrc[:])
                    else:
                        nc.vector.tensor_tensor(msb[b][:], msb[b][:], mrc[:],
                                                op=ALU.add)
                if pr % 2 == 1:
                    # batch b finished: mask cross-head blocks, AllReduce
                    mm = pmst.tile([128, D], FP, tag="mmsk", name=f"mm{l}_{b}")
                    nc.vector.tensor_tensor(
                        mm[:].rearrange("p (j v) -> p j v", v=128),
                        msb[b][:].rearrange("p (j v) -> p j v", v=128),
                        maskc[:].rearrange("p (j v) -> p j v", j=1)
                            .broadcast_to([128, 4, 128]),
                        op=ALU.mult)
                    cin = pdram.tile([128, D], FP, tag="cc_in")
                    cout = pdram.tile([128, D], FP, tag="cc_out")
                    nc.sync.dma_start(cin[:], mm[:])
                    nc.gpsimd.collective_compute(
                        "AllReduce", ALU.add,
                        ins=[cin[:].opt()],
                        outs=[cout[:].opt()],
                        replica_groups=[list(range(NCORES))],
                    )
                    mar[b] = pmst.tile([128, D], FH, tag="mar",
                                       name=f"mar{l}_{b}")
                    nc.gpsimd.dma_start(mar[b][:], cout[:])

            # ---- Q projection (feature-major, wide) + unit-norm ----
            qt = pacts.tile([128, DC, T], FH, tag="act")
            for dc4 in range(DC):
                bq_ap = bcol[:, COL_BQ + dc4:COL_BQ + dc4 + 1]
                psQ = psW.tile([128, T], FP, tag="w", name="psQ")
                for b in range(B):
                    for dc in range(DC):
                        nc.tensor.matmul(
                            psQ[:, b * SC:(b + 1) * SC],
                            wt[:, dc, dc4 * 128:(dc4 + 1) * 128],
                            ht[:, dc, b * SC:(b + 1) * SC],
                            start=(dc == 0), stop=(dc == DC - 1),
                        )
                qsb = pscr.tile([128, T], FP, tag="scr")
                nc.scalar.activation(qsb[:], psQ[:], AF.Identity, bias=bq_ap)
                sqq = pscr.tile([128, T], FR, tag="scr")
                nc.scalar.activation(sqq[:], psQ[:], AF.Square, bias=bq_ap)
                ninv = pstat.tile([2, T], FR, tag="qninv")
                for b in range(B):
                    ssp = psN.tile([2, SC], FP, tag="n", name="ssq")
                    nc.tensor.matmul(ssp[:], blk2[:],
                                     sqq[:, b * SC:(b + 1) * SC],
                                     start=True, stop=True)
                    nrm = pstat.tile([2, SC], FP, tag="qn")
                    nc.scalar.activation(nrm[:], ssp[:], AF.Sqrt)
                    with nc.allow_low_precision(reason="fp32r bcast operand"):
                        nc.vector.reciprocal(ninv[:, b * SC:(b + 1) * SC],
                                             nrm[:])
                bc = psW.tile([128, T], FP, tag="w", name="bcq")
                for b in range(B):
                    nc.tensor.matmul(bc[:, b * SC:(b + 1) * SC], blk2t[:],
                                     ninv[:, b * SC:(b + 1) * SC],
                                     start=True, stop=True)
                nc.vector.tensor_tensor(qt[:, dc4, :], qsb[:], bc[:],
                                        op=ALU.mult)

            w1t = pw1.tile([128, DC, FF], FH, tag="w1")
            for dc in range(DC):
                nc.sync.dma_start(w1t[:, dc, :], w1[l, dc])
            w2t = pw2.tile([128, FFC, D], FH, tag="w2")
            for fc in range(FFC):
                nc.sync.dma_start(w2t[:, fc, :], w2[l, fc])

            # ---- per-batch: O^T -> Wo -> FFN -> LayerNorm ----
            ot = pacts.tile([128, DC, T], FH, tag="act")
            h2 = pacts.tile([128, DC, T], FH, tag="act")
            htn = pacts.tile([128, DC, T], FH, tag="act")
            tsb = pacts.tile([128, DC, T], FH, tag="act")
            for b in range(B):
                bs = slice(b * SC, (b + 1) * SC)
                for dc4 in range(DC):
                    ps = psN.tile([128, SC], FP, tag="n", name="psO")
                    nc.tensor.matmul(
                        ps[:],
                        mar[b][:, dc4 * 128:(dc4 + 1) * 128],
                        qt[:, dc4, bs],
                        start=True, stop=True,
                    )
                    nc.scalar.activation(ot[:, dc4, bs], ps[:], AF.Copy)
                for dc4 in range(DC):
                    bo_ap = bcol[:, COL_BO + dc4:COL_BO + dc4 + 1]
                    ps = psN.tile([128, SC], FP, tag="n", name="psH")
                    for dc in range(DC):
                        nc.tensor.matmul(
                            ps[:],
                            wt[:, dc, 3 * D + dc4 * 128:3 * D + (dc4 + 1) * 128],
                            ot[:, dc, bs],
                            start=(dc == 0), stop=(dc == DC - 1),
                        )
                    tb = pscr.tile([128, T], FP, tag="scr")
                    nc.scalar.activation(tb[:, 0:SC], ps[:], AF.Identity,
                                         bias=bo_ap)
                    nc.vector.tensor_tensor(h2[:, dc4, bs], tb[:, 0:SC],
                                            ht[:, dc4, bs], op=ALU.add)
                fsball = pfsall.tile([128, FFC, SC], FH, tag="fsball")
                for fc in range(FFC):
                    fps = psW.tile([128, T], FP, tag="w", name="fps")
                    for dc in range(DC):
                        nc.tensor.matmul(
                            fps[:, 0:SC],
                            w1t[:, dc, fc * 128:(fc + 1) * 128],
                            h2[:, dc, bs],
                            start=(dc == 0), stop=(dc == DC - 1),
                        )
                    b1_ap = bcol[:, COL_B1 + fc:COL_B1 + fc + 1]
                    if fc % 2 == 0:
                        nc.scalar.activation(fsball[:, fc, :], fps[:, 0:SC],
                                             AF.Relu, bias=b1_ap)
                    else:
                        nc.vector.tensor_scalar(fsball[:, fc, :], fps[:, 0:SC],
                                                b1_ap, 0.0, ALU.add, ALU.max)
                # t = h2 + f2 + b2; LN stats over D (partition dim)
                sm = psW.tile([128, T], FP, tag="w", name="smean")
                for dc4 in range(DC):
                    gps1 = psN.tile([128, SC], FP, tag="n", name="gps1")
                    for fc in range(FFC):
                        nc.tensor.matmul(
                            gps1[:],
                            w2t[:, fc, dc4 * 128:(dc4 + 1) * 128],
                            fsball[:, fc, :],
                            start=(fc == 0), stop=(fc == FFC - 1),
                        )
                    tb = pscr.tile([128, T], FP, tag="scr")
                    nc.scalar.activation(tb[:, 0:SC], gps1[:], AF.Identity,
                                         bias=bcol[:, COL_B2 + dc4:COL_B2 + dc4 + 1])
                    nc.vector.tensor_tensor(
                        tsb[:, dc4, bs], tb[:, 0:SC], h2[:, dc4, bs],
                        op=ALU.add)
                    nc.tensor.matmul(sm[:1, 0:SC], invD[:],
                                     tsb[:, dc4, bs],
                                     start=(dc4 == 0), stop=(dc4 == DC - 1))
                    sqt = pfsb.tile([128, SC], FH, tag="fsb")
                    nc.vector.tensor_tensor(
                        sqt[:], tsb[:, dc4, bs],
                        tsb[:, dc4, bs], op=ALU.mult)
                    nc.tensor.matmul(sm[:1, SC:T], invD[:], sqt[:],
                                     start=(dc4 == 0), stop=(dc4 == DC - 1))
                smc = pstat.tile([1, SC], FP, tag="lns")
                nc.vector.tensor_copy(smc[:], sm[:1, 0:SC])
                rr = pstat.tile([1, SC], FP, tag="lns")
                nc.vector.tensor_tensor(rr[:], smc[:], smc[:], op=ALU.mult)
                nc.vector.tensor_tensor(rr[:], sm[:1, SC:T], rr[:],
                                        op=ALU.subtract)
                nc.scalar.activation(rr[:], rr[:], AF.Sqrt, bias=epsc[:, 0:1])
                u = pstat.tile([1, SC], FR, tag="lnu")
                with nc.allow_low_precision(reason="fp32r bcast operand"):
                    nc.vector.reciprocal(u[:], rr[:])
                w = pstat.tile([1, SC], FR, tag="lnu")
                nc.vector.tensor_tensor(w[:], smc[:], u[:], op=ALU.mult)
                bcu = psW.tile([128, T], FP, tag="w", name="bcu")
                nc.tensor.matmul(bcu[:, 0:SC], ones1r[:], u[:],
                                 start=True, stop=True)
                nc.tensor.matmul(bcu[:, SC:T], ones1r[:], w[:],
                                 start=True, stop=True)
                for dc4 in range(DC):
                    a1 = pscr.tile([128, T], FP, tag="scr")
                    nc.vector.tensor_tensor(
                        a1[:, 0:SC], tsb[:, dc4, bs], bcu[:, 0:SC],
                        op=ALU.mult)
                    nc.vector.tensor_tensor(a1[:, SC:T], a1[:, 0:SC],
                                            bcu[:, SC:T], op=ALU.subtract)
                    nc.vector.tensor_scalar(
                        htn[:, dc4, bs], a1[:, SC:T],
                        bcol[:, COL_G2 + dc4:COL_G2 + dc4 + 1],
                        bcol[:, COL_BE2 + dc4:COL_BE2 + dc4 + 1],
                        ALU.mult, ALU.add)
            ht = htn

        # ---- final classifier: out[b, v, s] = (h @ Wfc^T + bfc)^T ----
        bf = pbias.tile([128, VC], FP, tag="bfc")
        nc.sync.dma_start(bf[:], bfc_cols[:])
        for g in range(V // 512):
            wf = pwfc.tile([128, DC, 512], FH, tag="wfc")
            for dc in range(DC):
                nc.sync.dma_start(wf[:, dc, :], wfc[g, dc])
            for vci in range(4):
                vc = 4 * g + vci
                ps = psW.tile([128, T], FP, tag="w", name="psC")
                for b in range(B):
                    for dc in range(DC):
                        nc.tensor.matmul(
                            ps[:, b * SC:(b + 1) * SC],
                            wf[:, dc, vci * 128:(vci + 1) * 128],
                            ht[:, dc, b * SC:(b + 1) * SC],
                            start=(dc == 0), stop=(dc == DC - 1),
                        )
                osb = pscr.tile([128, T], FP, tag="scr")
                nc.scalar.activation(osb[:], ps[:], AF.Identity,
                                     bias=bf[:, vc:vc + 1])
                for b in range(B):
                    nc.sync.dma_start(out[b, vc * 128:(vc + 1) * 128, :],
                                      osb[:, b * SC:(b + 1) * SC])

    nc.compile()
    return nc


_CACHE = {}


def _get_nc(n_layers):
    if n_layers not in _CACHE:
        _CACHE[n_layers] = build(n_layers)
    return _CACHE[n_layers]


def prepare_maps(input, emb, Wq, bq, Wk, bk, Wv, bv, Wo, bo, W1, b1, W2, b2,
                 g2, be2, Wfc, bfc, n_layers):
    L = n_layers
    f32 = np.float32
    f16 = np.float16

    def t(x):
        return np.ascontiguousarray(np.asarray(x, dtype=f32))

    wqkvo = np.empty((L, DC, 128, 4 * D), f16)
    w1p = np.empty((L, DC, 128, FF), f16)
    w2p = np.empty((L, FFC, 128, D), f16)
    bias_cols = np.empty((L, 128, N_BCOLS), f32)
    bias_rows = np.empty((L, 1, 2 * D), f16)
    for l in range(L):
        cat = np.concatenate(
            [t(Wq[l]).T, t(Wk[l]).T, t(Wv[l]).T, t(Wo[l]).T], axis=1)
        wqkvo[l] = cat.reshape(DC, 128, 4 * D).astype(f16)
        w1p[l] = t(W1[l]).T.reshape(DC, 128, FF).astype(f16)
        w2p[l] = t(W2[l]).T.reshape(FFC, 128, D).astype(f16)
        bias_cols[l, :, COL_BQ:COL_BQ + 4] = t(bq[l]).reshape(4, 128).T
        bias_cols[l, :, COL_BO:COL_BO + 4] = t(bo[l]).reshape(4, 128).T
        bias_cols[l, :, COL_B2:COL_B2 + 4] = t(b2[l]).reshape(4, 128).T
        bias_cols[l, :, COL_G2:COL_G2 + 4] = t(g2[l]).reshape(4, 128).T
        bias_cols[l, :, COL_BE2:COL_BE2 + 4] = t(be2[l]).reshape(4, 128).T
        bias_cols[l, :, COL_B1:COL_B1 + 16] = t(b1[l]).reshape(16, 128).T
        bias_rows[l, 0, :D] = t(bk[l])
        bias_rows[l, 0, D:] = t(bv[l])
    wfcp = np.ascontiguousarray(
        t(Wfc).T.reshape(DC, 128, V // 512, 512).transpose(2, 0, 1, 3)
    ).astype(f16)
    bfcp = np.ascontiguousarray(t(bfc).reshape(VC, 128).T)

    cblk2 = np.zeros((128, 2), f32)
    cblk2[0:64, 0] = 1.0
    cblk2[64:128, 1] = 1.0
    cblk2t = np.ascontiguousarray(cblk2.T)
    cones1 = np.ones((1, 128), f16)
    cinvd = np.full((128, 1), 1.0 / D, f16)
    ceps = np.full((1, 128), EPS, f32)
    cmask = np.zeros((128, 128), f32)
    cmask[0:64, 0:64] = 1.0
    cmask[64:128, 64:128] = 1.0

    emb_np = t(emb)
    ids = np.asarray(input)
    in_maps = []
    for c in range(NCORES):
        hc = emb_np[ids[:, c * SC:(c + 1) * SC]]        # [B, SC, D]
        h0c = np.ascontiguousarray(
            hc.transpose(2, 0, 1).reshape(DC, 128, T)).astype(f16)
        in_maps.append({
            "h0": h0c,
            "wqkvo": wqkvo, "w1": w1p, "w2": w2p, "wfc": wfcp,
            "bias_cols": bias_cols, "bias_rows": bias_rows,
            "bfc_cols": bfcp,
            "cblk2": cblk2, "cblk2t": cblk2t, "cones1": cones1,
            "cones1r": cones1.astype(f32),
            "cinvd": cinvd, "ceps": ceps, "cmask": cmask,
        })
    return in_maps


def kernel(**inputs):
    n_layers = int(os.environ.get("KERNEL_LAYERS", "6"))
    nc = _get_nc(n_layers)
    in_maps = prepare_maps(n_layers=n_layers, **inputs)
    res = run_bass_kernel_spmd(nc, in_maps, core_ids=list(range(NCORES)))
    out = np.concatenate([res.results[c]["out"] for c in range(NCORES)], axis=2)
    return out
